# revision 1
# baseline (speedup 1.0000x reference)
"""Trainium2 Bass kernel for nn_BehaviorVelocity (velocity-driven swap sim + smoothing).

Sharding: data-parallel over batch B=16 across 8 cores (2 images/core, no collectives).

Layout per 512x512 image: partition p holds rows 4p..4p+3 as free-dim "slots".
Padded field = [128, 6 slots, 514 cols]; all spatial shifts are free-dim AP offsets.

State per image:
  V0, V1 : [P, 6, 514] f32   velocity planes (vy, vx)
  PK1    : [P, 6, 514] u32   channels 1,2,5,6 as fp8-e4m3 bytes
  PK2    : [P, 6, 514] u16   channels 7,0 as fp8 bytes (E = channel 0 = hi byte)
Masks are bf16 (DVE 2x tensor_tensor / 4x tensor_scalar modes).

swaps field (bf16, rebuilt each iter): 0..7 = taken (gather direction),
8 = free non-empty, 9 = free AND empty. The per-step match is
  m1 = want_a * (swaps >= 8);  M8 = m1 * (swaps_view == 9)
which folds the reference's (swaps==-1) & dir_swap==-1 & dir_empty checks.

Sector selection replicates floor(8*arccos-angle+0.5) via threshold compares in
the squared domain:  vx <= K*(mag+0.001)  <=>  (vx<=0) or (vx^2 <= K^2*magp2)
with magp2 = m2 + 0.002*mag + 1e-6.

Custom DVE ops fuse the compare chains (registered into concourse.dve_ops at
import; the per-NEFF DVE table ships them automatically).
"""

import sys

sys.path.insert(0, "/opt/trn_rl_repo")

import numpy as np

import concourse.bacc as bacc
import concourse.bass as bass_mod
import concourse.mybir as mybir
from concourse.tile import TileContext
from concourse.bass_utils import run_bass_kernel_spmd

dt = mybir.dt
Alu = mybir.AluOpType
Act = mybir.ActivationFunctionType

P = 128          # partitions
S = 4            # row-slots per partition (512 rows / 128)
W = 512
Wp = W + 2       # 514 with x-halo cols
NB = 2           # batch images per core
NCORES = 8

_DY = [0, 1, 1, 1, 0, -1, -1, -1]
_DX = [1, 1, 0, -1, -1, -1, 0, 1]

K0SQ = float(np.cos(np.pi / 8) ** 2)      # 0.85355339059
K1SQ = float(np.cos(3 * np.pi / 8) ** 2)  # 0.14644660941

PK1_CH = [1, 2, 5, 6]   # fp8 bytes of the u32 stream, lo..hi
PK2_CH = [7, 0]         # fp8 bytes of the u16 stream; E (ch 0) is the hi byte

REPEAT = 1  # profiling knob: emit the whole pipeline N times
# GPSIMD rules learned on HW: tensor_scalar on Pool is pathologically slow,
# and chains of dependent Pool ops cost ~100x their sim estimate. Pool is used
# only for isolated side-work whose consumers are on other engines.
POOL_INITS = True        # phase-C init copies on Pool (False: ACT)
POOL_AUX = True          # d1-3/s0m on Pool (False: DVE)
TIMING_VARIANT = None    # None | "nocust" (stock stand-ins for customs; timing only)

_cache = {}

# ---------------------------------------------------------------------------
# custom DVE ops (idempotent registration)
# ---------------------------------------------------------------------------

def _register_custom_ops():
    from concourse import dve_ops as DO
    from concourse.dve_spec import (
        Spec, Src0, Src1, Zero, One, C0, eq, ne, sq, maxx, lower,
        _has_src1 as has_src1,
    )
    from concourse.dve_uop import DveOpSpec

    if any(op.name == "BV_M2S" for op in DO.OPS):
        return {op.name: op for op in DO.OPS if op.name.startswith("BV_")}

    f32 = np.float32
    defs = [
        # m2 = vy^2 + vx^2
        ("BV_M2S", Spec(
            body=sq(Src0) + sq(Src1),
            reference=lambda in0, in1, s0, s1, imm2:
                (f32(in0) * f32(in0) + f32(in1) * f32(in1)).astype(f32))),
        # u_le = (vx <= 0) | (vx^2 <= T)
        ("BV_ULE", Spec(
            body=maxx(Src0 <= Zero, sq(Src0) <= Src1),
            reference=lambda in0, in1, s0, s1, imm2:
                np.maximum(f32(in0) <= 0, (f32(in0) * f32(in0)) <= f32(in1)).astype(f32))),
        # u_gt = (vx <= 0) & (vx^2 > T)
        ("BV_UGT", Spec(
            body=(Src0 <= Zero) * (sq(Src0) > Src1),
            reference=lambda in0, in1, s0, s1, imm2:
                ((f32(in0) <= 0) * ((f32(in0) * f32(in0)) > f32(in1))).astype(f32))),
        # en = (m2 > th2) & (E != 1)
        ("BV_ENW", Spec(
            body=(Src0 > C0) * ne(Src1, One),
            reference=lambda in0, in1, s0, s1, imm2:
                ((f32(in0) > f32(s0)) * (f32(in1) != 1.0)).astype(f32))),
        # s1 = (vy < 0) & en
        ("BV_S1M", Spec(
            body=(Src0 < Zero) * Src1,
            reference=lambda in0, in1, s0, s1, imm2:
                ((f32(in0) < 0) * f32(in1)).astype(f32))),
        # M8 = m1 * (swaps_view == 9)
        ("BV_M8E", Spec(
            body=Src0 * eq(Src1, C0),
            reference=lambda in0, in1, s0, s1, imm2:
                (f32(in0) * (f32(in1) == f32(s0))).astype(f32))),
    ]

    ops = {}
    for name, spec in defs:
        row = DO._CUSTOM_DVE_ROW_BASE + len(DO.OPS)
        assert row < 0x20, "custom DVE op rows exhausted"
        shas = {}
        for ver in ("v3", "v4"):
            tmp = DveOpSpec(name=name, opcode=row, uops=lower(spec, ver=ver),
                            rd1_en=has_src1(spec))
            shas[ver] = tmp.sha(ver)
        op = DO.DveOp(name, spec, False, shas)
        DO.OPS.append(op)
        DO.CUSTOM_DVE_SPECS[name] = spec
        DO._SUB_OPCODE_FOR_NAME[name] = row
        ops[name] = op
    return ops


_OPS = _register_custom_ops()


def _emit_custom(nc, opname, out, in0, in1, s0=0.0):
    if TIMING_VARIANT == "nocust":
        # shape-identical stand-in with known cost (timing-only; wrong numerics)
        nc.vector.tensor_scalar(out=out, in0=in0, scalar1=1.0, scalar2=None,
                                op0=mybir.AluOpType.is_ge)
        return
    nc.vector._custom_dve(_OPS[opname], out=out, in0=in0, in1=in1, s0=s0)


def _interior(t):
    return t[:, 1:1 + S, 1:1 + W]


def _view(t, dy, dx):
    return t[:, 1 + dy:1 + S + dy, 1 + dx:1 + W + dx]


class _Emit:
    def __init__(self, nk):
        self.nk = nk  # 3x3 conv kernel (already /18)
        nc = self.nc = bacc.Bacc()
        self.win = nc.declare_dram_parameter("w", [NB, 8, 512, 512], dt.float32, isOutput=False)
        self.wout = nc.declare_dram_parameter("o", [NB, 8, 512, 512], dt.float32, isOutput=True)

    # ---------- fp8 views ----------

    def _pk1_views(self, t):
        b = t[:].bitcast(dt.float8e4)                       # [P, 6, 4*Wp]
        b = b.rearrange("p s (c four) -> p s c four", four=4)
        return [b[:, :, :, i] for i in range(4)]

    def _pk2_views(self, t):
        b = t[:].bitcast(dt.float8e4)                       # [P, 6, 2*Wp]
        b = b.rearrange("p s (c two) -> p s c two", two=2)
        return [b[:, :, :, i] for i in range(2)]

    def E_view(self, pk2):
        return self._pk2_views(pk2)[1]                      # [P, 6, Wp] fp8

    # ---------- build ----------

    def build(self):
        nc = self.nc
        with TileContext(nc) as tc:
            self.tc = tc
            with (
                tc.tile_pool(name="pconst", bufs=1) as pconst,
                tc.tile_pool(name="pV", bufs=6) as pV,      # f32 plane slots (+staging)
                tc.tile_pool(name="pPK1", bufs=3) as pPK1,
                tc.tile_pool(name="pPK2", bufs=3) as pPK2,
                tc.tile_pool(name="pSw", bufs=1) as pSw,
                tc.tile_pool(name="pM8", bufs=1) as pM8,
                tc.tile_pool(name="pW", bufs=4) as pW,
                tc.tile_pool(name="pT", bufs=7) as pT,
                tc.tile_pool(name="pF", bufs=2) as pF,
                tc.tile_pool(name="pB", bufs=2) as pB,
            ):
                self.pV, self.pPK1, self.pPK2 = pV, pPK1, pPK2
                self.pSw, self.pM8, self.pW = pSw, pM8, pW
                self.pT, self.pF, self.pB = pT, pF, pB
                self.cvals = pconst.tile([P, 10, 4], dt.bfloat16, tag="cvals", name="cvals")
                for v in range(10):
                    nc.vector.memset(self.cvals[:, v:v + 1, :], v)
                # one flat pipelined image stream (repeat boundaries overlap
                # exactly like image boundaries): image i+1's loads are emitted
                # after image i's FIRST iteration, spreading across iter 2
                imgs = [b for _r in range(REPEAT) for b in range(NB)]
                st = self.image_load(imgs[0])
                for i, b in enumerate(imgs):
                    self.image_iter(st, 0)
                    nxt = (self.image_load(imgs[i + 1])
                           if i + 1 < len(imgs) else None)
                    self.image_iter(st, 1)
                    self.image_final(b, st)
                    st = nxt
        nc.compile()
        return nc

    def cval(self, v):
        return self.cvals[:, v:v + 1, 0:1].to_broadcast([P, S, W])

    # ---------- halo helpers ----------

    def fill_xcols(self, t, slots=slice(1, 5), engine=None):
        nc = self.nc
        e = engine or nc.vector
        if e is nc.scalar:
            e.copy(out=t[:, slots, 0:1], in_=t[:, slots, W:W + 1])
            e.copy(out=t[:, slots, Wp - 1:Wp], in_=t[:, slots, 1:2])
        else:
            e.tensor_copy(out=t[:, slots, 0:1], in_=t[:, slots, W:W + 1])
            e.tensor_copy(out=t[:, slots, Wp - 1:Wp], in_=t[:, slots, 1:2])

    def fill_xcol_side(self, t, dx, slots=slice(1, 5), engine=None):
        nc = self.nc
        e = engine or nc.vector
        if dx > 0:
            e.tensor_copy(out=t[:, slots, Wp - 1:Wp], in_=t[:, slots, 1:2])
        elif dx < 0:
            e.tensor_copy(out=t[:, slots, 0:1], in_=t[:, slots, W:W + 1])

    def fill_yhalo(self, t, hi, zero_edge=False, dma=None):
        nc = self.nc
        dma = dma or nc.scalar
        if hi:
            if zero_edge:
                nc.vector.memset(t[:, 5], 0)
            dma.dma_start(out=t[0:P - 1, 5], in_=t[1:P, 1])
            if not zero_edge:
                dma.dma_start(out=t[P - 1:P, 5], in_=t[0:1, 1])
        else:
            if zero_edge:
                nc.vector.memset(t[:, 0], 0)
            dma.dma_start(out=t[1:P, 0], in_=t[0:P - 1, 4])
            if not zero_edge:
                dma.dma_start(out=t[0:1, 0], in_=t[P - 1:P, 4])

    def fill_halos(self, t, engine=None, dma=None):
        self.fill_xcols(t, engine=engine)
        self.fill_yhalo(t, hi=True, dma=dma)
        self.fill_yhalo(t, hi=False, dma=dma)

    # ---------- DRAM loads ----------

    def _load_padded_f32(self, b, c, t, dma=None):
        nc = self.nc
        dma = dma or nc.sync
        d = self.win[b, c].rearrange("(p k) x -> p k x", k=S)  # [128, 4, 512]
        dma.dma_start(out=t[:, 1:1 + S, 1:1 + W], in_=d)
        dma.dma_start(out=t[1:P, 0, 1:1 + W], in_=d[0:P - 1, S - 1, :])
        dma.dma_start(out=t[0:1, 0, 1:1 + W], in_=d[P - 1:P, S - 1, :])
        dma.dma_start(out=t[0:P - 1, 5, 1:1 + W], in_=d[1:P, 0, :])
        dma.dma_start(out=t[P - 1:P, 5, 1:1 + W], in_=d[0:1, 0, :])

    def image_load(self, b):
        nc = self.nc

        def stage(c, view):
            # interior-only: halos are filled once per packed tile afterwards
            stg = self.pV.tile([P, 6, Wp], dt.float32, tag="Vp", name="stg")
            d = self.win[b, c].rearrange("(p k) x -> p k x", k=S)
            nc.sync.dma_start(out=_interior(stg), in_=d)
            nc.scalar.copy(out=view[:, 1:1 + S, 1:1 + W], in_=_interior(stg))

        # PK2 first (E gates phase A), then V planes (direct loads), then PK1
        PK2 = self.pPK2.tile([P, 6, Wp], dt.uint16, tag="PK2", name="PK2")
        v_lo, v_hi = self._pk2_views(PK2)
        stage(PK2_CH[1], v_hi)   # E channel first: it gates phase A
        stage(PK2_CH[0], v_lo)
        self.fill_halos(PK2, engine=nc.gpsimd, dma=nc.sync)
        V0 = self.pV.tile([P, 6, Wp], dt.float32, tag="Vp", name="V0")
        V1 = self.pV.tile([P, 6, Wp], dt.float32, tag="Vp", name="V1")
        for t, c in ((V0, 3), (V1, 4)):
            self._load_padded_f32(b, c, t, dma=nc.gpsimd)
            self.fill_xcols(t, slots=slice(0, 6), engine=nc.scalar)
        PK1 = self.pPK1.tile([P, 6, Wp], dt.uint32, tag="PK1", name="PK1")
        for view, c in zip(self._pk1_views(PK1), PK1_CH):
            stage(c, view)
        self.fill_halos(PK1, engine=nc.gpsimd, dma=nc.sync)
        return {"V0": V0, "V1": V1, "PK1": PK1, "PK2": PK2}

    # ---------- per-iteration ----------

    def phase_A(self, st, thresh_sq):
        nc = self.nc
        vy = _interior(st["V0"])
        vx = _interior(st["V1"])
        E_int = _interior(self.E_view(st["PK2"]))

        # swaps = 8 + (E == 0); fill halos
        swaps = self.pSw.tile([P, 6, Wp], dt.bfloat16, tag="swaps", name="swaps")
        nc.vector.tensor_scalar(out=_interior(swaps), in0=E_int, scalar1=0.0,
                                scalar2=8.0, op0=Alu.is_equal, op1=Alu.add)
        self.fill_halos(swaps, dma=nc.sync)

        m2 = self.pF.tile([P, S, W], dt.float32, tag="f32t", name="m2")
        _emit_custom(nc, "BV_M2S", m2[:], vy, vx)
        mp = self.pF.tile([P, S, W], dt.float32, tag="f32t", name="mp")
        nc.scalar.activation(mp[:], m2[:], Act.Sqrt)
        nc.vector.scalar_tensor_tensor(out=mp[:], in0=mp[:], scalar=0.002, in1=m2[:],
                                       op0=Alu.mult, op1=Alu.add)      # magp2 - 1e-6

        def bt(name):
            return self.pT.tile([P, S, W], dt.bfloat16, tag="bft", name=name)

        # en first: it is m2's last consumer, freeing its pF slot before T0/T1
        en = bt("en")
        _emit_custom(nc, "BV_ENW", en[:], m2[:], E_int, s0=float(thresh_sq))
        T0 = self.pF.tile([P, S, W], dt.float32, tag="f32t", name="T0")
        nc.scalar.activation(T0[:], mp[:], Act.Copy, bias=1e-6 * K0SQ, scale=K0SQ)
        T1 = mp  # in place: mp's last reader is T0's activation
        nc.scalar.activation(T1[:], mp[:], Act.Copy, bias=1e-6 * K1SQ, scale=K1SQ)

        s1m = bt("s1m")
        _emit_custom(nc, "BV_S1M", s1m[:], vy, en[:])
        s0m = bt("s0m")
        _aux = nc.gpsimd if POOL_AUX else nc.vector
        _aux.tensor_tensor(out=s0m[:], in0=en[:], in1=s1m[:], op=Alu.subtract)

        want = [None] * 8

        def emit_w(a, f0, f1):
            wt = self.pW.tile([P, S, W], dt.bfloat16, tag="want", name=f"w{a}")
            nc.vector.tensor_tensor(out=wt[:], in0=f0[:], in1=f1[:], op=Alu.mult)
            want[a] = wt

        # slot-frugal order; d2/d3/nu0 in place, w0/w4 consume u0/u3
        u0, u1 = bt("u0"), bt("u1")
        _emit_custom(nc, "BV_ULE", u0[:], vx, T0[:])
        _emit_custom(nc, "BV_ULE", u1[:], vx, T1[:])
        d1 = bt("d1")
        _aux.tensor_tensor(out=d1[:], in0=u0[:], in1=u1[:], op=Alu.subtract)
        nu0 = u0  # in place: u0 <- 1 - u0 (after d1 read the original)
        nc.vector.tensor_scalar(out=nu0[:], in0=u0[:], scalar1=-1.0, scalar2=1.0,
                                op0=Alu.mult, op1=Alu.add)
        emit_w(0, nu0, en)            # frees u0
        u2 = bt("u2")
        _emit_custom(nc, "BV_UGT", u2[:], vx, T1[:])
        d2 = u1  # in place: u1 <- u1 - u2
        _aux.tensor_tensor(out=d2[:], in0=u1[:], in1=u2[:], op=Alu.subtract)
        u3 = bt("u3")
        _emit_custom(nc, "BV_UGT", u3[:], vx, T0[:])
        d3 = u2  # in place: u2 <- u2 - u3
        _aux.tensor_tensor(out=d3[:], in0=u2[:], in1=u3[:], op=Alu.subtract)
        emit_w(4, u3, en)             # frees u3 and en
        emit_w(1, d1, s0m)
        emit_w(7, d1, s1m)
        emit_w(2, d2, s0m)
        emit_w(6, d2, s1m)
        emit_w(3, d3, s0m)
        emit_w(5, d3, s1m)
        return want, swaps

    def phase_B(self, want, swaps):
        nc = self.nc
        for a in range(8):
            dy, dx = _DY[a], _DX[a]
            a4 = (a + 4) % 8
            dy4, dx4 = -dy, -dx
            E1 = self.pB.tile([P, S, W], dt.bfloat16, tag="bstep", name="E1")
            nc.vector.tensor_scalar(out=E1[:], in0=_interior(swaps), scalar1=8.0,
                                    scalar2=None, op0=Alu.is_ge)
            m1 = E1  # in place: m1 = want_a * E1
            nc.vector.tensor_tensor(out=m1[:], in0=want[a][:], in1=E1[:], op=Alu.mult)
            M8 = self.pM8.tile([P, 6, Wp], dt.uint16, tag="M8", name="M8")
            _emit_custom(nc, "BV_M8E", _interior(M8), m1[:], _view(swaps, dy, dx), s0=9.0)
            # halos of M8 on the (dy4, dx4) view side
            self.fill_xcol_side(M8, dx4, engine=nc.vector)
            if dy4 > 0:
                self.fill_yhalo(M8, hi=True, dma=nc.sync)
            elif dy4 < 0:
                self.fill_yhalo(M8, hi=False, dma=nc.sync)
            nc.vector.copy_predicated(out=_interior(swaps), mask=_interior(M8),
                                      data=self.cval(a))
            nc.vector.copy_predicated(out=_interior(swaps), mask=_view(M8, dy4, dx4),
                                      data=self.cval(a4))
            # refresh swaps halos needed by the next step's view
            if a < 7:
                dyn, dxn = _DY[a + 1], _DX[a + 1]
                self.fill_xcol_side(swaps, dxn, engine=nc.gpsimd)
                if dyn > 0:
                    self.fill_yhalo(swaps, hi=True, dma=nc.sync)
                elif dyn < 0:
                    self.fill_yhalo(swaps, hi=False, dma=nc.sync)
        return swaps

    def phase_C_prep(self, st):
        """Allocate + init the next-state tiles. Emitted BEFORE phase B so the
        Pool copies run during B (in-order queues would otherwise pin them
        behind B's per-step xcol ops)."""
        nc = self.nc
        V0, V1, PK1, PK2 = st["V0"], st["V1"], st["PK1"], st["PK2"]
        nV0 = self.pV.tile([P, 6, Wp], dt.float32, tag="Vp", name="nV0")
        nV1 = self.pV.tile([P, 6, Wp], dt.float32, tag="Vp", name="nV1")
        nPK1 = self.pPK1.tile([P, 6, Wp], dt.uint32, tag="PK1", name="nPK1")
        nPK2 = self.pPK2.tile([P, 6, Wp], dt.uint16, tag="PK2", name="nPK2")
        if POOL_INITS:
            nc.gpsimd.tensor_copy(out=_interior(nPK1), in_=_interior(PK1))
            nc.gpsimd.tensor_copy(out=_interior(nPK2), in_=_interior(PK2))
            nc.gpsimd.tensor_copy(out=_interior(nV0), in_=_interior(V0))
            nc.gpsimd.tensor_copy(out=_interior(nV1), in_=_interior(V1))
        else:
            nc.scalar.copy(out=_interior(nPK1), in_=_interior(PK1))
            nc.scalar.copy(out=_interior(nPK2), in_=_interior(PK2))
            nc.scalar.copy(out=_interior(nV0), in_=_interior(V0))
            nc.scalar.copy(out=_interior(nV1), in_=_interior(V1))
        return nV0, nV1, nPK1, nPK2

    def phase_C(self, st, swaps, news, last):
        nc = self.nc
        V0, V1, PK1, PK2 = st["V0"], st["V1"], st["PK1"], st["PK2"]
        nV0, nV1, nPK1, nPK2 = news
        for a in range(8):
            dy, dx = _DY[a], _DX[a]
            equ = self.pB.tile([P, S, W], dt.uint16, tag="bstep", name="equ")
            nc.vector.tensor_scalar(out=equ[:], in0=_interior(swaps), scalar1=float(a),
                                    scalar2=None, op0=Alu.is_equal)
            nc.vector.copy_predicated(out=_interior(nPK1), mask=equ[:],
                                      data=_view(PK1, dy, dx))
            nc.vector.copy_predicated(out=_interior(nPK2), mask=equ[:],
                                      data=_view(PK2, dy, dx))
            nc.vector.copy_predicated(out=_interior(nV0), mask=equ[:],
                                      data=_view(V0, dy, dx))
            nc.vector.copy_predicated(out=_interior(nV1), mask=equ[:],
                                      data=_view(V1, dy, dx))
        # vel blend: nV = 0.5*(nV + V)  (Pool TT add + ACT halve; Pool
        # tensor_scalar is pathologically slow on HW - never use it)
        for nV, V in ((nV0, V0), (nV1, V1)):
            nc.gpsimd.tensor_tensor(out=_interior(nV), in0=_interior(nV),
                                    in1=_interior(V), op=Alu.add)
            nc.scalar.mul(_interior(nV), _interior(nV), 0.5)
        st["V0"], st["V1"], st["PK1"], st["PK2"] = nV0, nV1, nPK1, nPK2
        if not last:
            # halos needed by the next iteration's phase-C views
            self.fill_halos(nV0, engine=nc.scalar, dma=nc.sync)
            self.fill_halos(nV1, engine=nc.scalar, dma=nc.sync)
            self.fill_halos(nPK1, engine=nc.gpsimd, dma=nc.sync)
            self.fill_halos(nPK2, engine=nc.gpsimd, dma=nc.sync)

    def image_iter(self, st, n):
        thresh_sq = 1.0 if n == 0 else 4.0
        want, swaps = self.phase_A(st, thresh_sq)
        news = self.phase_C_prep(st)
        swaps = self.phase_B(want, swaps)
        self.phase_C(st, swaps, news, last=(n == 1))

    # ---------- final conv + stores ----------

    def image_final(self, b, st):
        nc = self.nc
        PK1, PK2 = st["PK1"], st["PK2"]
        nk = self.nk
        uniform = bool(np.allclose(nk, nk[0, 0]))
        assert uniform, "non-uniform neighbor_kernel not supported in this build"
        scale = float(nk[0, 0])

        for c, key in ((3, "V0"), (4, "V1")):
            V = st[key]
            # vel *= 0.95 on the interior; zero halos for zero-padded conv
            nc.scalar.mul(_interior(V), _interior(V), 0.95)
            nc.vector.memset(V[:, 1:5, 0:1], 0)
            nc.vector.memset(V[:, 1:5, Wp - 1:Wp], 0)
            self.fill_yhalo(V, hi=True, zero_edge=True)
            self.fill_yhalo(V, hi=False, zero_edge=True)
            # row sums over x into a padded tmp; zero y-edges; column sums
            tp = self.pV.tile([P, 6, Wp], dt.float32, tag="Vp", name="convtp")
            nc.gpsimd.tensor_tensor(out=_interior(tp), in0=_view(V, 0, -1),
                                    in1=_view(V, 0, 0), op=Alu.add)
            nc.vector.tensor_tensor(out=_interior(tp), in0=_interior(tp),
                                    in1=_view(V, 0, 1), op=Alu.add)
            self.fill_yhalo(tp, hi=True, zero_edge=True)
            self.fill_yhalo(tp, hi=False, zero_edge=True)
            acc = self.pF.tile([P, S, W], dt.float32, tag="f32t", name="acc")
            nc.vector.tensor_tensor(out=acc[:], in0=_view(tp, -1, 0),
                                    in1=_view(tp, 0, 0), op=Alu.add)
            nc.vector.tensor_tensor(out=acc[:], in0=acc[:], in1=_view(tp, 1, 0), op=Alu.add)
            half = self.pF.tile([P, S, W], dt.float32, tag="f32t", name="half")
            nc.scalar.mul(half[:], _interior(V), 0.5)
            nc.vector.scalar_tensor_tensor(out=acc[:], in0=acc[:], scalar=scale,
                                           in1=half[:], op0=Alu.mult, op1=Alu.add)
            nc.sync.dma_start(out=self.wout[b, c].rearrange("(p k) x -> p k x", k=S),
                              in_=acc[:])

        # payload stores: fp8 -> f32 staging -> DRAM (casts split ACT/Pool,
        # out-DMAs on the gpsimd queue so they don't block the next image's loads)
        all_views = list(zip(self._pk1_views(PK1), PK1_CH)) + \
                    list(zip(self._pk2_views(PK2), PK2_CH))
        for i, (view, c) in enumerate(all_views):
            stg = self.pV.tile([P, S, W], dt.float32, tag="Vp", name="ostg")
            if i % 2 == 0:
                nc.scalar.copy(out=stg[:], in_=view[:, 1:1 + S, 1:1 + W])
            else:
                nc.gpsimd.tensor_copy(out=stg[:], in_=view[:, 1:1 + S, 1:1 + W])
            dq = (nc.sync, nc.scalar, nc.gpsimd)[i % 3]
            dq.dma_start(out=self.wout[b, c].rearrange("(p k) x -> p k x", k=S),
                         in_=stg[:])


def _build(nk):
    return _Emit(nk).build()


def kernel(world, rand_movement=None, rand_interact=None, rand_element=None,
           neighbor_kernel=None, **_kw):
    world = np.ascontiguousarray(np.asarray(world, dtype=np.float32))
    nk = np.asarray(neighbor_kernel, dtype=np.float32).reshape(3, 3) / 18.0
    key = nk.tobytes()
    nc = _cache.get(key)
    if nc is None:
        nc = _cache[key] = _build(nk)
    in_maps = [{"w": world[NB * i:NB * (i + 1)]} for i in range(NCORES)]
    res = run_bass_kernel_spmd(nc, in_maps, list(range(NCORES))).results
    return np.concatenate([r["o"] for r in res], axis=0)



# revision 67
# speedup vs baseline: 1.8812x; 1.8812x over previous
"""Trainium2 Bass kernel for nn_BehaviorVelocity (velocity-driven swap sim + smoothing).

Sharding: data-parallel over batch B=16 across 8 cores (2 images/core, no collectives).

Layout per 512x512 image: partition p holds rows 4p..4p+3 as free-dim "slots".
Padded field = [128, 6 slots, 514 cols]; all spatial shifts are free-dim AP offsets.

State per image:
  V0, V1 : [P, 6, 514] f32   velocity planes (vy, vx)
  PK1    : [P, 6, 514] u32   channels 1,2,5,6 as fp8-e4m3 bytes
  PK2    : [P, 6, 514] u16   channels 7,0 as fp8 bytes (E = channel 0 = hi byte)
Masks are bf16 (DVE 2x tensor_tensor / 4x tensor_scalar modes).

swaps field (bf16, rebuilt each iter): 0..7 = taken (gather direction),
8 = free non-empty, 9 = free AND empty. The per-step match is
  m1 = want_a * (swaps >= 8);  M8 = m1 * (swaps_view == 9)
which folds the reference's (swaps==-1) & dir_swap==-1 & dir_empty checks.

Sector selection replicates floor(8*arccos-angle+0.5) via threshold compares in
the squared domain:  vx <= K*(mag+0.001)  <=>  (vx<=0) or (vx^2 <= K^2*magp2)
with magp2 = m2 + 0.002*mag + 1e-6.

Custom DVE ops fuse the compare chains (registered into concourse.dve_ops at
import; the per-NEFF DVE table ships them automatically).
"""

import sys

sys.path.insert(0, "/opt/trn_rl_repo")

import numpy as np

import concourse.bacc as bacc
import concourse.bass as bass_mod
import concourse.mybir as mybir
from concourse.tile import TileContext
from concourse.bass_utils import run_bass_kernel_spmd

dt = mybir.dt
Alu = mybir.AluOpType
Act = mybir.ActivationFunctionType

P = 128          # partitions
S = 4            # row-slots per partition (512 rows / 128)
W = 512
Wp = W + 2       # 514 with x-halo cols
NB = 2           # batch images per core
NCORES = 8

_DY = [0, 1, 1, 1, 0, -1, -1, -1]
_DX = [1, 1, 0, -1, -1, -1, 0, 1]

K0SQ = float(np.cos(np.pi / 8) ** 2)      # 0.85355339059
K1SQ = float(np.cos(3 * np.pi / 8) ** 2)  # 0.14644660941

PK1_CH = [1, 2, 5, 6]   # fp8 bytes of the u32 stream, lo..hi
PK2_CH = [7, 0]         # fp8 bytes of the u16 stream; E (ch 0) is the hi byte

REPEAT = 1  # profiling knob: emit the whole pipeline N times
# GPSIMD rules learned on HW: tensor_scalar on Pool is pathologically slow,
# and chains of dependent Pool ops cost ~100x their sim estimate. Pool is used
# only for isolated side-work whose consumers are on other engines.
POOL_INITS = False       # phase-C init copies now fixed on DVE/ACT (see phase_C_prep)
POOL_AUX = False         # d1-3/s0m on DVE (bf16 TT 2x mode ~1.3us vs ~5us Pool)
TIMING_VARIANT = None    # None | "nocust" (stock stand-ins for customs; timing only)

_cache = {}

# ---------------------------------------------------------------------------
# custom DVE ops (idempotent registration)
# ---------------------------------------------------------------------------

def _register_custom_ops():
    from concourse import dve_ops as DO
    from concourse.dve_spec import (
        Spec, Src0, Src1, Zero, One, C0, C1, eq, ne, sq, maxx, lower,
        _has_src1 as has_src1,
    )
    from concourse.dve_uop import DveOpSpec

    if any(op.name == "BV_EQE" for op in DO.OPS):
        return {op.name: op for op in DO.OPS if op.name.startswith("BV_")}
    assert not any(op.name.startswith("BV_") for op in DO.OPS), \
        "stale partial BV_ op registration"

    f32 = np.float32
    defs = [
        # m2 = vy^2 + vx^2
        ("BV_M2S", Spec(
            body=sq(Src0) + sq(Src1),
            reference=lambda in0, in1, s0, s1, imm2:
                (f32(in0) * f32(in0) + f32(in1) * f32(in1)).astype(f32))),
        # u_le = (vx <= 0) | (vx^2 <= T)
        ("BV_ULE", Spec(
            body=maxx(Src0 <= Zero, sq(Src0) <= Src1),
            reference=lambda in0, in1, s0, s1, imm2:
                np.maximum(f32(in0) <= 0, (f32(in0) * f32(in0)) <= f32(in1)).astype(f32))),
        # u_gt = (vx <= 0) & (vx^2 > T)
        ("BV_UGT", Spec(
            body=(Src0 <= Zero) * (sq(Src0) > Src1),
            reference=lambda in0, in1, s0, s1, imm2:
                ((f32(in0) <= 0) * ((f32(in0) * f32(in0)) > f32(in1))).astype(f32))),
        # en = (m2 > th2) & (E != 1)
        ("BV_ENW", Spec(
            body=(Src0 > C0) * ne(Src1, One),
            reference=lambda in0, in1, s0, s1, imm2:
                ((f32(in0) > f32(s0)) * (f32(in1) != 1.0)).astype(f32))),
        # s1 = (vy < 0) & en
        ("BV_S1M", Spec(
            body=(Src0 < Zero) * Src1,
            reference=lambda in0, in1, s0, s1, imm2:
                ((f32(in0) < 0) * f32(in1)).astype(f32))),
        # M8 = m1 * (swaps_view == 9)
        ("BV_M8E", Spec(
            body=Src0 * eq(Src1, C0),
            reference=lambda in0, in1, s0, s1, imm2:
                (f32(in0) * (f32(in1) == f32(s0))).astype(f32))),
    ]
    # band count: vx>0 -> [vx^2<=T]; vx<=0 -> 2-[vx^2<=T]  (s0=2), so that
    # c = CBD(T0)+CBD(T1) equals u0+u1+u2+u3 of the ULE/UGT formulation.
    _L = sq(Src0) <= Src1
    defs.append(("BV_CBD", Spec(
        body=_L + (Src0 <= Zero) * (C0 - (_L + _L)),
        reference=lambda in0, in1, s0, s1, imm2:
            (lambda L: (L + (f32(in0) <= 0) * (f32(s0) - 2 * L)).astype(f32))(
                ((f32(in0) * f32(in0)) <= f32(in1)).astype(f32)))))
    # M8 = (A' == a) * (swaps_view == 9): the full per-step match in one op
    # (A' is invalidated at target-claimed cells, so no separate free check)
    defs.append(("BV_EQE", Spec(
        body=eq(Src0, C0) * eq(Src1, C1),
        reference=lambda in0, in1, s0, s1, imm2:
            ((f32(in0) == f32(s0)) * (f32(in1) == f32(s1))).astype(f32))))

    ops = {}
    for name, spec in defs:
        row = DO._CUSTOM_DVE_ROW_BASE + len(DO.OPS)
        assert row < 0x20, "custom DVE op rows exhausted"
        shas = {}
        for ver in ("v3", "v4"):
            tmp = DveOpSpec(name=name, opcode=row, uops=lower(spec, ver=ver),
                            rd1_en=has_src1(spec))
            shas[ver] = tmp.sha(ver)
        op = DO.DveOp(name, spec, False, shas)
        DO.OPS.append(op)
        DO.CUSTOM_DVE_SPECS[name] = spec
        DO._SUB_OPCODE_FOR_NAME[name] = row
        ops[name] = op
    return ops


_OPS = _register_custom_ops()


def _emit_custom(nc, opname, out, in0, in1, s0=0.0, s1=0.0):
    if TIMING_VARIANT == "nocust":
        # shape-identical stand-in with known cost (timing-only; wrong numerics)
        nc.vector.tensor_scalar(out=out, in0=in0, scalar1=1.0, scalar2=None,
                                op0=mybir.AluOpType.is_ge)
        return
    nc.vector._custom_dve(_OPS[opname], out=out, in0=in0, in1=in1, s0=s0, s1=s1)


def _interior(t):
    return t[:, 1:1 + S, 1:1 + W]


def _view(t, dy, dx):
    return t[:, 1 + dy:1 + S + dy, 1 + dx:1 + W + dx]


class _Emit:
    def __init__(self, nk):
        self.nk = nk  # 3x3 conv kernel (already /18)
        nc = self.nc = bacc.Bacc()
        self.win = nc.declare_dram_parameter("w", [NB, 8, 512, 512], dt.float32, isOutput=False)
        self.wout = nc.declare_dram_parameter("o", [NB, 8, 512, 512], dt.float32, isOutput=True)

    # ---------- fp8 views ----------

    def _pk1_views(self, t):
        b = t[:].bitcast(dt.float8e4)                       # [P, 6, 4*Wp]
        b = b.rearrange("p s (c four) -> p s c four", four=4)
        return [b[:, :, :, i] for i in range(4)]

    def _pk2_views(self, t):
        b = t[:].bitcast(dt.float8e4)                       # [P, 6, 2*Wp]
        b = b.rearrange("p s (c two) -> p s c two", two=2)
        return [b[:, :, :, i] for i in range(2)]

    def E_view(self, pk2):
        return self._pk2_views(pk2)[1]                      # [P, 6, Wp] fp8

    # ---------- build ----------

    def build(self):
        nc = self.nc
        with TileContext(nc) as tc:
            self.tc = tc
            with (
                tc.tile_pool(name="pconst", bufs=1) as pconst,
                tc.tile_pool(name="pV", bufs=7) as pV,      # f32 plane slots (+staging)
                tc.tile_pool(name="pPK1", bufs=3) as pPK1,
                tc.tile_pool(name="pPK2", bufs=4) as pPK2,
                tc.tile_pool(name="pSw", bufs=1) as pSw,
                tc.tile_pool(name="pM8", bufs=1) as pM8,
                tc.tile_pool(name="pT", bufs=6) as pT,
                tc.tile_pool(name="pF", bufs=2) as pF,
                tc.tile_pool(name="pB", bufs=2) as pB,
            ):
                self.pV, self.pPK1, self.pPK2 = pV, pPK1, pPK2
                self.pSw, self.pM8 = pSw, pM8
                self.pT, self.pF, self.pB = pT, pF, pB
                self.cvals = pconst.tile([P, 16, 4], dt.bfloat16, tag="cvals", name="cvals")
                for v in range(16):
                    nc.vector.memset(self.cvals[:, v:v + 1, :], v)
                # one flat pipelined image stream (repeat boundaries overlap
                # exactly like image boundaries): image i+1's loads are emitted
                # after image i's FIRST iteration, spreading across iter 2
                imgs = [b for _r in range(REPEAT) for b in range(NB)]
                st = self.image_load(imgs[0])
                for i, b in enumerate(imgs):
                    self.image_iter(st, 0)
                    nxt = (self.image_load(imgs[i + 1])
                           if i + 1 < len(imgs) else None)
                    self.image_iter(st, 1)
                    self.image_final(b, st)
                    st = nxt
        nc.compile()
        return nc

    def cval(self, v):
        return self.cvals[:, v:v + 1, 0:1].to_broadcast([P, S, W])

    # ---------- halo helpers ----------

    def fill_xcols(self, t, slots=slice(1, 5), engine=None):
        nc = self.nc
        e = engine or nc.vector
        if e is nc.scalar:
            e.copy(out=t[:, slots, 0:1], in_=t[:, slots, W:W + 1])
            e.copy(out=t[:, slots, Wp - 1:Wp], in_=t[:, slots, 1:2])
        else:
            e.tensor_copy(out=t[:, slots, 0:1], in_=t[:, slots, W:W + 1])
            e.tensor_copy(out=t[:, slots, Wp - 1:Wp], in_=t[:, slots, 1:2])

    def fill_xcol_side(self, t, dx, slots=slice(1, 5), engine=None):
        nc = self.nc
        e = engine or nc.vector
        if dx > 0:
            e.tensor_copy(out=t[:, slots, Wp - 1:Wp], in_=t[:, slots, 1:2])
        elif dx < 0:
            e.tensor_copy(out=t[:, slots, 0:1], in_=t[:, slots, W:W + 1])

    def fill_yhalo(self, t, hi, zero_edge=False, dma=None, wrap_dma=None,
                   dma2=None):
        # bulk (127-partition) and wrap (1-partition) transfers go on separate
        # queues so the DMA latencies overlap instead of serializing; when dma2
        # is given the bulk is further halved across (dma, dma2) since a full
        # 127-row SBUF-to-SBUF transfer runs ~4us on one engine.
        nc = self.nc
        dma = dma or nc.scalar
        wrap_dma = wrap_dma or nc.gpsimd
        H = 64
        if hi:
            if zero_edge:
                nc.vector.memset(t[:, 5], 0)
            if not zero_edge:
                wrap_dma.dma_start(out=t[P - 1:P, 5], in_=t[0:1, 1])
            if dma2 is not None:
                dma.dma_start(out=t[0:H, 5], in_=t[1:H + 1, 1])
                dma2.dma_start(out=t[H:P - 1, 5], in_=t[H + 1:P, 1])
            else:
                dma.dma_start(out=t[0:P - 1, 5], in_=t[1:P, 1])
        else:
            if zero_edge:
                nc.vector.memset(t[:, 0], 0)
            if not zero_edge:
                wrap_dma.dma_start(out=t[0:1, 0], in_=t[P - 1:P, 4])
            if dma2 is not None:
                dma.dma_start(out=t[1:H, 0], in_=t[0:H - 1, 4])
                dma2.dma_start(out=t[H:P, 0], in_=t[H - 1:P - 1, 4])
            else:
                dma.dma_start(out=t[1:P, 0], in_=t[0:P - 1, 4])

    def fill_halos(self, t, engine=None, dma=None, wrap_dma=None, dma2=None):
        self.fill_xcols(t, engine=engine)
        self.fill_yhalo(t, hi=True, dma=dma, wrap_dma=wrap_dma, dma2=dma2)
        self.fill_yhalo(t, hi=False, dma=dma, wrap_dma=wrap_dma, dma2=dma2)

    # ---------- DRAM loads ----------

    def _load_padded_f32(self, b, c, t, dma=None):
        nc = self.nc
        dma = dma or nc.sync
        d = self.win[b, c].rearrange("(p k) x -> p k x", k=S)  # [128, 4, 512]
        dma.dma_start(out=t[:, 1:1 + S, 1:1 + W], in_=d)
        dma.dma_start(out=t[1:P, 0, 1:1 + W], in_=d[0:P - 1, S - 1, :])
        dma.dma_start(out=t[0:1, 0, 1:1 + W], in_=d[P - 1:P, S - 1, :])
        dma.dma_start(out=t[0:P - 1, 5, 1:1 + W], in_=d[1:P, 0, :])
        dma.dma_start(out=t[P - 1:P, 5, 1:1 + W], in_=d[0:1, 0, :])

    def image_load(self, b, first=False):
        nc = self.nc
        # Steady state: all load DMAs on gpsimd so sync/scalar stay reserved
        # for the halo refreshes of the overlapped previous-image compute.
        # For the pipeline-head image there is no overlapped compute, so
        # spread loads across sync/scalar/gpsimd to cut the cold-start chain.
        sdma = nc.scalar if first else nc.gpsimd
        vdma = (nc.sync, nc.gpsimd) if first else (nc.gpsimd, nc.gpsimd)

        def stage(c, view):
            # interior-only: halos are filled once per packed tile afterwards
            stg = self.pV.tile([P, 6, Wp], dt.float32, tag="Vp", name="stg")
            d = self.win[b, c].rearrange("(p k) x -> p k x", k=S)
            sdma.dma_start(out=_interior(stg), in_=d)
            nc.scalar.copy(out=view[:, 1:1 + S, 1:1 + W], in_=_interior(stg))

        # PK2 first (E gates phase A), then V planes (direct loads), then PK1
        PK2 = self.pPK2.tile([P, 6, Wp], dt.uint16, tag="PK2", name="PK2")
        v_lo, v_hi = self._pk2_views(PK2)
        stage(PK2_CH[1], v_hi)   # E channel first: it gates phase A
        stage(PK2_CH[0], v_lo)
        self.fill_halos(PK2, engine=nc.gpsimd, dma=nc.gpsimd)
        V0 = self.pV.tile([P, 6, Wp], dt.float32, tag="Vp", name="V0")
        V1 = self.pV.tile([P, 6, Wp], dt.float32, tag="Vp", name="V1")
        for (t, c), dq in zip(((V0, 3), (V1, 4)), vdma):
            self._load_padded_f32(b, c, t, dma=dq)
            self.fill_xcols(t, slots=slice(0, 6), engine=nc.scalar)
        PK1 = self.pPK1.tile([P, 6, Wp], dt.uint32, tag="PK1", name="PK1")
        for view, c in zip(self._pk1_views(PK1), PK1_CH):
            stage(c, view)
        self.fill_halos(PK1, engine=nc.gpsimd, dma=nc.gpsimd)
        return {"V0": V0, "V1": V1, "PK1": PK1, "PK2": PK2}

    # ---------- per-iteration ----------

    def phase_A(self, st, thresh_sq):
        nc = self.nc
        vy = _interior(st["V0"])
        vx = _interior(st["V1"])
        E_int = _interior(self.E_view(st["PK2"]))

        # swaps = 8 + (E == 0), derived from E so no swaps halo DMAs are
        # needed. Emitted as interior + xcols + halo-slot pieces: the interior
        # only needs PK2's interior (ready right after the previous gather),
        # so phase A is not blocked on PK2's halo-fill DMAs.
        Ev = self.E_view(st["PK2"])
        swaps = self.pSw.tile([P, 6, Wp], dt.bfloat16, tag="swaps", name="swaps")
        nc.vector.tensor_scalar(out=_interior(swaps), in0=E_int, scalar1=0.0,
                                scalar2=8.0, op0=Alu.is_equal, op1=Alu.add)
        self.fill_xcols(swaps, engine=nc.vector)

        m2 = self.pF.tile([P, S, W], dt.float32, tag="f32t", name="m2")
        _emit_custom(nc, "BV_M2S", m2[:], vy, vx)
        mp = self.pF.tile([P, S, W], dt.float32, tag="f32t", name="mp")
        nc.scalar.activation(mp[:], m2[:], Act.Sqrt)
        nc.vector.scalar_tensor_tensor(out=mp[:], in0=mp[:], scalar=0.002, in1=m2[:],
                                       op0=Alu.mult, op1=Alu.add)      # magp2 - 1e-6

        # swaps halo slots, emitted here so the PK2 halo-fill DMA latency
        # hides behind the m2/magp2 chain (only phase B's first view needs it)
        for sl in (slice(0, 1), slice(5, 6)):
            nc.vector.tensor_scalar(out=swaps[:, sl], in0=Ev[:, sl], scalar1=0.0,
                                    scalar2=8.0, op0=Alu.is_equal, op1=Alu.add)

        def bt(name):
            return self.pT.tile([P, S, W], dt.bfloat16, tag="bft", name=name)

        # en first: it is m2's last consumer, freeing its pF slot before T0/T1
        en = bt("en")
        _emit_custom(nc, "BV_ENW", en[:], m2[:], E_int, s0=float(thresh_sq))
        T0 = self.pF.tile([P, S, W], dt.float32, tag="f32t", name="T0")
        nc.scalar.activation(T0[:], mp[:], Act.Copy, bias=1e-6 * K0SQ, scale=K0SQ)
        T1 = mp  # in place: mp's last reader is T0's activation
        nc.scalar.activation(T1[:], mp[:], Act.Copy, bias=1e-6 * K1SQ, scale=K1SQ)

        s1m = bt("s1m")
        _emit_custom(nc, "BV_S1M", s1m[:], vy, en[:])

        # sector-index field A' instead of 8 one-hot want tiles:
        #   c  = u0+u1+u2+u3  (== |angle| band 0..4, same compare primitives)
        #   A  = (c + s1m*(8-2c)) mod 8        (vy<0 mirrors the sector)
        #   A' = A + 15*(1-en)                 (sentinel >= 15 for gated cells)
        # bf16 arithmetic on small ints is exact, so A' == a replicates the
        # want_a masks bit-for-bit at ~2/3 the DVE cost, and frees the pW pool.
        # band count via the fused CBD custom: c = CBD(T0) + CBD(T1)
        # (bit-identical to the u0+u1+u2+u3 ULE/UGT formulation)
        cb0, cb1 = bt("cb0"), bt("cb1")
        _emit_custom(nc, "BV_CBD", cb0[:], vx, T0[:], s0=2.0)
        _emit_custom(nc, "BV_CBD", cb1[:], vx, T1[:], s0=2.0)
        c = cb0  # in place
        nc.vector.tensor_tensor(out=c[:], in0=cb0[:], in1=cb1[:], op=Alu.add)
        g = cb1  # g = 8 - 2c
        nc.vector.tensor_scalar(out=g[:], in0=c[:], scalar1=-2.0, scalar2=8.0,
                                op0=Alu.mult, op1=Alu.add)
        t4 = bt("t4")  # t4 = s1m * (8-2c)
        nc.vector.tensor_tensor(out=t4[:], in0=s1m[:], in1=g[:], op=Alu.mult)
        A = g  # A = c + t4; the c==0 & vy<0 case gives 8, which must wrap
        # to 0: A <- (A <= 7) * A   (no mod op in the DVE ISA)
        nc.vector.tensor_tensor(out=A[:], in0=c[:], in1=t4[:], op=Alu.add)
        t5 = t4  # t5 = (A <= 7)
        nc.vector.tensor_scalar(out=t5[:], in0=A[:], scalar1=7.0, scalar2=None,
                                op0=Alu.is_le)
        A2 = c  # A2 = A * (A <= 7)
        nc.vector.tensor_tensor(out=A2[:], in0=A[:], in1=t5[:], op=Alu.mult)
        es = s1m  # es = 15*(1-en)
        nc.vector.tensor_scalar(out=es[:], in0=en[:], scalar1=-15.0, scalar2=15.0,
                                op0=Alu.mult, op1=Alu.add)
        nc.vector.tensor_tensor(out=A2[:], in0=A2[:], in1=es[:], op=Alu.add)
        return A2, swaps

    def phase_B(self, Ap, swaps):
        nc = self.nc
        # Per step: M8 = (A' == a) * (view(swaps) == 9) in ONE custom. The
        # mover-side free check (swaps >= 8) is unnecessary because a cell can
        # only mover-match at its own unique sector step; cells claimed as
        # TARGETS get A' invalidated (set to 15) with the same view-mask CP
        # that writes swaps, so they can never mover-match later.
        for a in range(8):
            dy, dx = _DY[a], _DX[a]
            a4 = (a + 4) % 8
            dy4, dx4 = -dy, -dx
            M8 = self.pM8.tile([P, 6, Wp], dt.uint16, tag="M8", name="M8")
            _emit_custom(nc, "BV_EQE", _interior(M8), Ap[:], _view(swaps, dy, dx),
                         s0=float(a), s1=9.0)
            # halos of M8 on the (dy4, dx4) view side, bulk split sync/scalar
            self.fill_xcol_side(M8, dx4, engine=nc.vector)
            if dy4 > 0:
                self.fill_yhalo(M8, hi=True, dma=nc.sync, wrap_dma=nc.gpsimd,
                                dma2=nc.scalar)
            elif dy4 < 0:
                self.fill_yhalo(M8, hi=False, dma=nc.sync, wrap_dma=nc.gpsimd,
                                dma2=nc.scalar)
            nc.vector.copy_predicated(out=_interior(swaps), mask=_interior(M8),
                                      data=self.cval(a))
            nc.vector.copy_predicated(out=_interior(swaps), mask=_view(M8, dy4, dx4),
                                      data=self.cval(a4))
            if a < 7:
                nc.vector.copy_predicated(out=Ap[:], mask=_view(M8, dy4, dx4),
                                          data=self.cval(15))
            # refresh swaps halos needed by the next step's view
            if a < 7:
                dyn, dxn = _DY[a + 1], _DX[a + 1]
                self.fill_xcol_side(swaps, dxn, engine=nc.vector)
                if dyn > 0:
                    self.fill_yhalo(swaps, hi=True, dma=nc.sync,
                                    wrap_dma=nc.gpsimd, dma2=nc.scalar)
                elif dyn < 0:
                    self.fill_yhalo(swaps, hi=False, dma=nc.sync,
                                    wrap_dma=nc.gpsimd, dma2=nc.scalar)
        return swaps

    def phase_C_prep(self, st):
        """Allocate + init the next-state tiles. Emitted BEFORE phase B so the
        Pool copies run during B (in-order queues would otherwise pin them
        behind B's per-step xcol ops)."""
        nc = self.nc
        V0, V1, PK1, PK2 = st["V0"], st["V1"], st["PK1"], st["PK2"]
        nV0 = self.pV.tile([P, 6, Wp], dt.float32, tag="Vp", name="nV0")
        nV1 = self.pV.tile([P, 6, Wp], dt.float32, tag="Vp", name="nV1")
        nPK1 = self.pPK1.tile([P, 6, Wp], dt.uint32, tag="PK1", name="nPK1")
        nPK2 = self.pPK2.tile([P, 6, Wp], dt.uint16, tag="PK2", name="nPK2")
        # nPK1 MUST be bit-exact: u32-packed fp8 bytes through the ACT f32
        # datapath lose low mantissa bits (ch1/ch2 corruption) - DMA it.
        # u16 (< 2^24) and f32 round-trip exactly through ACT.
        nc.sync.dma_start(out=_interior(nPK1), in_=_interior(PK1))
        nc.scalar.copy(out=_interior(nPK2), in_=_interior(PK2))
        nc.scalar.copy(out=_interior(nV0), in_=_interior(V0))
        nc.scalar.copy(out=_interior(nV1), in_=_interior(V1))
        return nV0, nV1, nPK1, nPK2

    def phase_C(self, st, swaps, news, last):
        nc = self.nc
        V0, V1, PK1, PK2 = st["V0"], st["V1"], st["PK1"], st["PK2"]
        nV0, nV1, nPK1, nPK2 = news
        for a in range(8):
            dy, dx = _DY[a], _DX[a]
            equ = self.pB.tile([P, S, W], dt.uint16, tag="bstep", name="equ")
            nc.vector.tensor_scalar(out=equ[:], in0=_interior(swaps), scalar1=float(a),
                                    scalar2=None, op0=Alu.is_equal)
            nc.vector.copy_predicated(out=_interior(nPK2), mask=equ[:],
                                      data=_view(PK2, dy, dx))
            nc.vector.copy_predicated(out=_interior(nPK1), mask=equ[:],
                                      data=_view(PK1, dy, dx))
            nc.vector.copy_predicated(out=_interior(nV0), mask=equ[:],
                                      data=_view(V0, dy, dx))
            nc.vector.copy_predicated(out=_interior(nV1), mask=equ[:],
                                      data=_view(V1, dy, dx))
        if not last:
            # PK halos issued BEFORE the blend: their DMAs fly while the DVE
            # does the blend, so the next iteration's swaps halo-slot init
            # (which reads PK2's halo) doesn't stall. PK2 gathers/fills come
            # first - they are the most urgent.
            self.fill_halos(nPK2, engine=nc.gpsimd, dma=nc.sync, wrap_dma=nc.gpsimd,
                            dma2=nc.scalar)
            self.fill_halos(nPK1, engine=nc.gpsimd, dma=nc.sync, wrap_dma=nc.gpsimd,
                            dma2=nc.scalar)
        # vel blend: nV = 0.5*(nV + V) on Vector (Pool/ACT variants measured
        # slower end-to-end: the blend sits on the iteration's critical tail).
        # On the last iteration the final vel *= 0.95 decay is folded in.
        bs = 0.475 if last else 0.5
        for nV, V in ((nV0, V0), (nV1, V1)):
            nc.vector.tensor_tensor(out=_interior(nV), in0=_interior(nV),
                                    in1=_interior(V), op=Alu.add)
            nc.vector.tensor_scalar(out=_interior(nV), in0=_interior(nV),
                                    scalar1=bs, scalar2=None, op0=Alu.mult)
        st["V0"], st["V1"], st["PK1"], st["PK2"] = nV0, nV1, nPK1, nPK2
        if not last:
            # V halos aren't read until the next phase C - fill last
            self.fill_halos(nV0, engine=nc.scalar, dma=nc.sync, wrap_dma=nc.gpsimd,
                            dma2=nc.scalar)
            self.fill_halos(nV1, engine=nc.scalar, dma=nc.sync, wrap_dma=nc.gpsimd,
                            dma2=nc.scalar)

    def image_iter(self, st, n):
        thresh_sq = 1.0 if n == 0 else 4.0
        Ap, swaps = self.phase_A(st, thresh_sq)
        news = self.phase_C_prep(st)
        swaps = self.phase_B(Ap, swaps)
        self.phase_C(st, swaps, news, last=(n == 1))

    # ---------- final conv + stores ----------

    def image_final(self, b, st):
        nc = self.nc
        PK1, PK2 = st["PK1"], st["PK2"]
        nk = self.nk
        uniform = bool(np.allclose(nk, nk[0, 0]))
        assert uniform, "non-uniform neighbor_kernel not supported in this build"
        scale = float(nk[0, 0])

        # payload stores FIRST: PK planes are final right after phase C, and
        # storing early releases their pool buffers for the next image's prep
        all_views = list(zip(self._pk1_views(PK1), PK1_CH)) + \
                    list(zip(self._pk2_views(PK2), PK2_CH))
        for i, (view, c) in enumerate(all_views):
            stg = self.pV.tile([P, S, W], dt.float32, tag="Vp", name="ostg")
            if i % 2 == 0:
                nc.scalar.copy(out=stg[:], in_=view[:, 1:1 + S, 1:1 + W])
            else:
                nc.vector.tensor_copy(out=stg[:], in_=view[:, 1:1 + S, 1:1 + W])
            dq = (nc.gpsimd, nc.scalar)[i % 2]
            dq.dma_start(out=self.wout[b, c].rearrange("(p k) x -> p k x", k=S),
                         in_=stg[:])

        for c, key in ((3, "V0"), (4, "V1")):
            V = st[key]
            # (vel *= 0.95 is folded into the last blend as 0.475)
            nc.vector.memset(V[:, 1:5, 0:1], 0)
            nc.vector.memset(V[:, 1:5, Wp - 1:Wp], 0)
            self.fill_yhalo(V, hi=True, zero_edge=True, dma=nc.sync,
                            dma2=nc.scalar)
            self.fill_yhalo(V, hi=False, zero_edge=True, dma=nc.sync,
                            dma2=nc.scalar)
            # row sums over x into a padded tmp; zero y-edges; column sums
            tp = self.pV.tile([P, 6, Wp], dt.float32, tag="Vp", name="convtp")
            # row sums computed over ALL 6 slots (halo rows included): V's
            # halo slots hold valid (zero-edged) neighbor rows, so tp's halo
            # rows come out bit-identical to a DMA'd copy of the neighbor's
            # tp - and two DMA chains leave the final's critical tail.
            nc.gpsimd.tensor_tensor(out=tp[:, 0:6, 1:1 + W],
                                    in0=V[:, 0:6, 0:W],
                                    in1=V[:, 0:6, 1:1 + W], op=Alu.add)
            nc.vector.tensor_tensor(out=tp[:, 0:6, 1:1 + W],
                                    in0=tp[:, 0:6, 1:1 + W],
                                    in1=V[:, 0:6, 2:2 + W], op=Alu.add)
            acc = self.pF.tile([P, S, W], dt.float32, tag="f32t", name="acc")
            nc.gpsimd.tensor_tensor(out=acc[:], in0=_view(tp, -1, 0),
                                    in1=_view(tp, 0, 0), op=Alu.add)
            nc.vector.tensor_tensor(out=acc[:], in0=acc[:], in1=_view(tp, 1, 0), op=Alu.add)
            half = self.pF.tile([P, S, W], dt.float32, tag="f32t", name="half")
            nc.scalar.mul(half[:], _interior(V), 0.5)
            nc.vector.scalar_tensor_tensor(out=acc[:], in0=acc[:], scalar=scale,
                                           in1=half[:], op0=Alu.mult, op1=Alu.add)
            nc.scalar.dma_start(out=self.wout[b, c].rearrange("(p k) x -> p k x", k=S),
                                in_=acc[:])


def _build(nk):
    return _Emit(nk).build()


def kernel(world, rand_movement=None, rand_interact=None, rand_element=None,
           neighbor_kernel=None, **_kw):
    world = np.ascontiguousarray(np.asarray(world, dtype=np.float32))
    nk = np.asarray(neighbor_kernel, dtype=np.float32).reshape(3, 3) / 18.0
    key = nk.tobytes()
    nc = _cache.get(key)
    if nc is None:
        nc = _cache[key] = _build(nk)
    in_maps = [{"w": world[NB * i:NB * (i + 1)]} for i in range(NCORES)]
    res = run_bass_kernel_spmd(nc, in_maps, list(range(NCORES))).results
    return np.concatenate([r["o"] for r in res], axis=0)



# revision 69
# speedup vs baseline: 1.9651x; 1.0446x over previous
"""Trainium2 Bass kernel for nn_BehaviorVelocity (velocity-driven swap sim + smoothing).

Sharding: data-parallel over batch B=16 across 8 cores (2 images/core, no collectives).

Layout per 512x512 image: partition p holds rows 4p..4p+3 as free-dim "slots".
Padded field = [128, 6 slots, 514 cols]; all spatial shifts are free-dim AP offsets.

State per image:
  V0, V1 : [P, 6, 514] f32   velocity planes (vy, vx)
  PK1    : [P, 6, 514] u32   channels 1,2,5,6 as fp8-e4m3 bytes
  PK2    : [P, 6, 514] u16   channels 7,0 as fp8 bytes (E = channel 0 = hi byte)
Masks are bf16 (DVE 2x tensor_tensor / 4x tensor_scalar modes).

swaps field (bf16, rebuilt each iter): 0..7 = taken (gather direction),
8 = free non-empty, 9 = free AND empty. The per-step match is
  m1 = want_a * (swaps >= 8);  M8 = m1 * (swaps_view == 9)
which folds the reference's (swaps==-1) & dir_swap==-1 & dir_empty checks.

Sector selection replicates floor(8*arccos-angle+0.5) via threshold compares in
the squared domain:  vx <= K*(mag+0.001)  <=>  (vx<=0) or (vx^2 <= K^2*magp2)
with magp2 = m2 + 0.002*mag + 1e-6.

Custom DVE ops fuse the compare chains (registered into concourse.dve_ops at
import; the per-NEFF DVE table ships them automatically).
"""

import sys

sys.path.insert(0, "/opt/trn_rl_repo")

import numpy as np

import concourse.bacc as bacc
import concourse.bass as bass_mod
import concourse.mybir as mybir
from concourse.tile import TileContext
from concourse.bass_utils import run_bass_kernel_spmd

dt = mybir.dt
Alu = mybir.AluOpType
Act = mybir.ActivationFunctionType

P = 128          # partitions
S = 4            # row-slots per partition (512 rows / 128)
W = 512
Wp = W + 2       # 514 with x-halo cols
NB = 2           # batch images per core
NCORES = 8

_DY = [0, 1, 1, 1, 0, -1, -1, -1]
_DX = [1, 1, 0, -1, -1, -1, 0, 1]

K0SQ = float(np.cos(np.pi / 8) ** 2)      # 0.85355339059
K1SQ = float(np.cos(3 * np.pi / 8) ** 2)  # 0.14644660941

PK1_CH = [1, 2, 5, 6]   # fp8 bytes of the u32 stream, lo..hi
PK2_CH = [7, 0]         # fp8 bytes of the u16 stream; E (ch 0) is the hi byte

REPEAT = 1  # profiling knob: emit the whole pipeline N times
# GPSIMD rules learned on HW: tensor_scalar on Pool is pathologically slow,
# and chains of dependent Pool ops cost ~100x their sim estimate. Pool is used
# only for isolated side-work whose consumers are on other engines.
POOL_INITS = False       # phase-C init copies now fixed on DVE/ACT (see phase_C_prep)
POOL_AUX = False         # d1-3/s0m on DVE (bf16 TT 2x mode ~1.3us vs ~5us Pool)
TIMING_VARIANT = None    # None | "nocust" (stock stand-ins for customs; timing only)

_cache = {}

# ---------------------------------------------------------------------------
# custom DVE ops (idempotent registration)
# ---------------------------------------------------------------------------

def _register_custom_ops():
    from concourse import dve_ops as DO
    from concourse.dve_spec import (
        Spec, Src0, Src1, Zero, One, C0, C1, eq, ne, sq, maxx, lower,
        _has_src1 as has_src1,
    )
    from concourse.dve_uop import DveOpSpec

    if any(op.name == "BV_EQE" for op in DO.OPS):
        return {op.name: op for op in DO.OPS if op.name.startswith("BV_")}
    assert not any(op.name.startswith("BV_") for op in DO.OPS), \
        "stale partial BV_ op registration"

    f32 = np.float32
    defs = [
        # m2 = vy^2 + vx^2
        ("BV_M2S", Spec(
            body=sq(Src0) + sq(Src1),
            reference=lambda in0, in1, s0, s1, imm2:
                (f32(in0) * f32(in0) + f32(in1) * f32(in1)).astype(f32))),
        # u_le = (vx <= 0) | (vx^2 <= T)
        ("BV_ULE", Spec(
            body=maxx(Src0 <= Zero, sq(Src0) <= Src1),
            reference=lambda in0, in1, s0, s1, imm2:
                np.maximum(f32(in0) <= 0, (f32(in0) * f32(in0)) <= f32(in1)).astype(f32))),
        # u_gt = (vx <= 0) & (vx^2 > T)
        ("BV_UGT", Spec(
            body=(Src0 <= Zero) * (sq(Src0) > Src1),
            reference=lambda in0, in1, s0, s1, imm2:
                ((f32(in0) <= 0) * ((f32(in0) * f32(in0)) > f32(in1))).astype(f32))),
        # en = (m2 > th2) & (E != 1)
        ("BV_ENW", Spec(
            body=(Src0 > C0) * ne(Src1, One),
            reference=lambda in0, in1, s0, s1, imm2:
                ((f32(in0) > f32(s0)) * (f32(in1) != 1.0)).astype(f32))),
        # s1 = (vy < 0) & en
        ("BV_S1M", Spec(
            body=(Src0 < Zero) * Src1,
            reference=lambda in0, in1, s0, s1, imm2:
                ((f32(in0) < 0) * f32(in1)).astype(f32))),
        # M8 = m1 * (swaps_view == 9)
        ("BV_M8E", Spec(
            body=Src0 * eq(Src1, C0),
            reference=lambda in0, in1, s0, s1, imm2:
                (f32(in0) * (f32(in1) == f32(s0))).astype(f32))),
    ]
    # band count: vx>0 -> [vx^2<=T]; vx<=0 -> 2-[vx^2<=T]  (s0=2), so that
    # c = CBD(T0)+CBD(T1) equals u0+u1+u2+u3 of the ULE/UGT formulation.
    _L = sq(Src0) <= Src1
    defs.append(("BV_CBD", Spec(
        body=_L + (Src0 <= Zero) * (C0 - (_L + _L)),
        reference=lambda in0, in1, s0, s1, imm2:
            (lambda L: (L + (f32(in0) <= 0) * (f32(s0) - 2 * L)).astype(f32))(
                ((f32(in0) * f32(in0)) <= f32(in1)).astype(f32)))))
    # M8 = (A' == a) * (swaps_view == 9): the full per-step match in one op
    # (A' is invalidated at target-claimed cells, so no separate free check)
    defs.append(("BV_EQE", Spec(
        body=eq(Src0, C0) * eq(Src1, C1),
        reference=lambda in0, in1, s0, s1, imm2:
            ((f32(in0) == f32(s0)) * (f32(in1) == f32(s1))).astype(f32))))

    ops = {}
    for name, spec in defs:
        row = DO._CUSTOM_DVE_ROW_BASE + len(DO.OPS)
        assert row < 0x20, "custom DVE op rows exhausted"
        shas = {}
        for ver in ("v3", "v4"):
            tmp = DveOpSpec(name=name, opcode=row, uops=lower(spec, ver=ver),
                            rd1_en=has_src1(spec))
            shas[ver] = tmp.sha(ver)
        op = DO.DveOp(name, spec, False, shas)
        DO.OPS.append(op)
        DO.CUSTOM_DVE_SPECS[name] = spec
        DO._SUB_OPCODE_FOR_NAME[name] = row
        ops[name] = op
    return ops


_OPS = _register_custom_ops()


def _emit_custom(nc, opname, out, in0, in1, s0=0.0, s1=0.0):
    if TIMING_VARIANT == "nocust":
        # shape-identical stand-in with known cost (timing-only; wrong numerics)
        nc.vector.tensor_scalar(out=out, in0=in0, scalar1=1.0, scalar2=None,
                                op0=mybir.AluOpType.is_ge)
        return
    nc.vector._custom_dve(_OPS[opname], out=out, in0=in0, in1=in1, s0=s0, s1=s1)


def _interior(t):
    return t[:, 1:1 + S, 1:1 + W]


def _view(t, dy, dx):
    return t[:, 1 + dy:1 + S + dy, 1 + dx:1 + W + dx]


class _Emit:
    def __init__(self, nk):
        self.nk = nk  # 3x3 conv kernel (already /18)
        nc = self.nc = bacc.Bacc()
        self.win = nc.declare_dram_parameter("w", [NB, 8, 512, 512], dt.float32, isOutput=False)
        self.wout = nc.declare_dram_parameter("o", [NB, 8, 512, 512], dt.float32, isOutput=True)

    # ---------- fp8 views ----------

    def _pk1_views(self, t):
        b = t[:].bitcast(dt.float8e4)                       # [P, 6, 4*Wp]
        b = b.rearrange("p s (c four) -> p s c four", four=4)
        return [b[:, :, :, i] for i in range(4)]

    def _pk2_views(self, t):
        b = t[:].bitcast(dt.float8e4)                       # [P, 6, 2*Wp]
        b = b.rearrange("p s (c two) -> p s c two", two=2)
        return [b[:, :, :, i] for i in range(2)]

    def E_view(self, pk2):
        return self._pk2_views(pk2)[1]                      # [P, 6, Wp] fp8

    # ---------- build ----------

    def build(self):
        nc = self.nc
        with TileContext(nc) as tc:
            self.tc = tc
            with (
                tc.tile_pool(name="pconst", bufs=1) as pconst,
                tc.tile_pool(name="pV", bufs=7) as pV,      # f32 plane slots (+staging)
                tc.tile_pool(name="pPK1", bufs=3) as pPK1,
                tc.tile_pool(name="pPK2", bufs=4) as pPK2,
                tc.tile_pool(name="pSw", bufs=1) as pSw,
                tc.tile_pool(name="pM8", bufs=1) as pM8,
                tc.tile_pool(name="pT", bufs=6) as pT,
                tc.tile_pool(name="pF", bufs=2) as pF,
                tc.tile_pool(name="pB", bufs=2) as pB,
            ):
                self.pV, self.pPK1, self.pPK2 = pV, pPK1, pPK2
                self.pSw, self.pM8 = pSw, pM8
                self.pT, self.pF, self.pB = pT, pF, pB
                self.cvals = pconst.tile([P, 16, 4], dt.bfloat16, tag="cvals", name="cvals")
                for v in range(16):
                    nc.vector.memset(self.cvals[:, v:v + 1, :], v)
                # one flat pipelined image stream (repeat boundaries overlap
                # exactly like image boundaries): image i+1's loads are emitted
                # after image i's FIRST iteration, spreading across iter 2
                imgs = [b for _r in range(REPEAT) for b in range(NB)]
                st = self.image_load(imgs[0])
                for i, b in enumerate(imgs):
                    self.image_iter(st, 0)
                    nxt = (self.image_load(imgs[i + 1])
                           if i + 1 < len(imgs) else None)
                    self.image_iter(st, 1)
                    self.image_final(b, st)
                    st = nxt
        nc.compile()
        return nc

    def cval(self, v):
        return self.cvals[:, v:v + 1, 0:1].to_broadcast([P, S, W])

    # ---------- halo helpers ----------

    def fill_xcols(self, t, slots=slice(1, 5), engine=None):
        nc = self.nc
        e = engine or nc.vector
        if e is nc.scalar:
            e.copy(out=t[:, slots, 0:1], in_=t[:, slots, W:W + 1])
            e.copy(out=t[:, slots, Wp - 1:Wp], in_=t[:, slots, 1:2])
        else:
            e.tensor_copy(out=t[:, slots, 0:1], in_=t[:, slots, W:W + 1])
            e.tensor_copy(out=t[:, slots, Wp - 1:Wp], in_=t[:, slots, 1:2])

    def fill_xcol_side(self, t, dx, slots=slice(1, 5), engine=None):
        nc = self.nc
        e = engine or nc.vector
        if dx > 0:
            e.tensor_copy(out=t[:, slots, Wp - 1:Wp], in_=t[:, slots, 1:2])
        elif dx < 0:
            e.tensor_copy(out=t[:, slots, 0:1], in_=t[:, slots, W:W + 1])

    def fill_yhalo(self, t, hi, zero_edge=False, dma=None, wrap_dma=None,
                   dma2=None):
        # bulk (127-partition) and wrap (1-partition) transfers go on separate
        # queues so the DMA latencies overlap instead of serializing; when dma2
        # is given the bulk is further halved across (dma, dma2) since a full
        # 127-row SBUF-to-SBUF transfer runs ~4us on one engine.
        nc = self.nc
        dma = dma or nc.scalar
        wrap_dma = wrap_dma or nc.gpsimd
        H = 64
        if hi:
            if zero_edge:
                nc.vector.memset(t[:, 5], 0)
            if not zero_edge:
                wrap_dma.dma_start(out=t[P - 1:P, 5], in_=t[0:1, 1])
            if dma2 is not None:
                dma.dma_start(out=t[0:H, 5], in_=t[1:H + 1, 1])
                dma2.dma_start(out=t[H:P - 1, 5], in_=t[H + 1:P, 1])
            else:
                dma.dma_start(out=t[0:P - 1, 5], in_=t[1:P, 1])
        else:
            if zero_edge:
                nc.vector.memset(t[:, 0], 0)
            if not zero_edge:
                wrap_dma.dma_start(out=t[0:1, 0], in_=t[P - 1:P, 4])
            if dma2 is not None:
                dma.dma_start(out=t[1:H, 0], in_=t[0:H - 1, 4])
                dma2.dma_start(out=t[H:P, 0], in_=t[H - 1:P - 1, 4])
            else:
                dma.dma_start(out=t[1:P, 0], in_=t[0:P - 1, 4])

    def fill_halos(self, t, engine=None, dma=None, wrap_dma=None, dma2=None):
        self.fill_xcols(t, engine=engine)
        self.fill_yhalo(t, hi=True, dma=dma, wrap_dma=wrap_dma, dma2=dma2)
        self.fill_yhalo(t, hi=False, dma=dma, wrap_dma=wrap_dma, dma2=dma2)

    def fill_halos_yfirst(self, t, dma=None, wrap_dma=None, dma2=None):
        """Latency-lean variant: interior-width y-bulks first (they only need
        the tile's interior, not its xcols), then xcols for ALL 6 slots from
        the tile's own wrap columns on Vector. Corner values are bit-identical
        to fill_halos (neighbor's col 512 either way); two semaphore hops and
        the gpsimd xcol queue leave the critical chain."""
        nc = self.nc
        H = 64
        for hi in (True, False):
            if hi:
                (wrap_dma or nc.gpsimd).dma_start(out=t[P - 1:P, 5, 1:1 + W],
                                                  in_=t[0:1, 1, 1:1 + W])
                (dma or nc.sync).dma_start(out=t[0:H, 5, 1:1 + W],
                                           in_=t[1:H + 1, 1, 1:1 + W])
                (dma2 or nc.scalar).dma_start(out=t[H:P - 1, 5, 1:1 + W],
                                              in_=t[H + 1:P, 1, 1:1 + W])
            else:
                (wrap_dma or nc.gpsimd).dma_start(out=t[0:1, 0, 1:1 + W],
                                                  in_=t[P - 1:P, 4, 1:1 + W])
                (dma or nc.sync).dma_start(out=t[1:H, 0, 1:1 + W],
                                           in_=t[0:H - 1, 4, 1:1 + W])
                (dma2 or nc.scalar).dma_start(out=t[H:P, 0, 1:1 + W],
                                              in_=t[H - 1:P - 1, 4, 1:1 + W])
        self.fill_xcols(t, slots=slice(0, 6), engine=nc.vector)

    # ---------- DRAM loads ----------

    def _load_padded_f32(self, b, c, t, dma=None):
        nc = self.nc
        dma = dma or nc.sync
        d = self.win[b, c].rearrange("(p k) x -> p k x", k=S)  # [128, 4, 512]
        dma.dma_start(out=t[:, 1:1 + S, 1:1 + W], in_=d)
        dma.dma_start(out=t[1:P, 0, 1:1 + W], in_=d[0:P - 1, S - 1, :])
        dma.dma_start(out=t[0:1, 0, 1:1 + W], in_=d[P - 1:P, S - 1, :])
        dma.dma_start(out=t[0:P - 1, 5, 1:1 + W], in_=d[1:P, 0, :])
        dma.dma_start(out=t[P - 1:P, 5, 1:1 + W], in_=d[0:1, 0, :])

    def image_load(self, b, first=False):
        nc = self.nc
        # Steady state: all load DMAs on gpsimd so sync/scalar stay reserved
        # for the halo refreshes of the overlapped previous-image compute.
        # For the pipeline-head image there is no overlapped compute, so
        # spread loads across sync/scalar/gpsimd to cut the cold-start chain.
        sdma = nc.scalar if first else nc.gpsimd
        vdma = (nc.sync, nc.gpsimd) if first else (nc.gpsimd, nc.gpsimd)

        def stage(c, view):
            # interior-only: halos are filled once per packed tile afterwards
            stg = self.pV.tile([P, 6, Wp], dt.float32, tag="Vp", name="stg")
            d = self.win[b, c].rearrange("(p k) x -> p k x", k=S)
            sdma.dma_start(out=_interior(stg), in_=d)
            nc.scalar.copy(out=view[:, 1:1 + S, 1:1 + W], in_=_interior(stg))

        # PK2 first (E gates phase A), then V planes (direct loads), then PK1
        PK2 = self.pPK2.tile([P, 6, Wp], dt.uint16, tag="PK2", name="PK2")
        v_lo, v_hi = self._pk2_views(PK2)
        stage(PK2_CH[1], v_hi)   # E channel first: it gates phase A
        stage(PK2_CH[0], v_lo)
        self.fill_halos(PK2, engine=nc.gpsimd, dma=nc.gpsimd)
        V0 = self.pV.tile([P, 6, Wp], dt.float32, tag="Vp", name="V0")
        V1 = self.pV.tile([P, 6, Wp], dt.float32, tag="Vp", name="V1")
        for (t, c), dq in zip(((V0, 3), (V1, 4)), vdma):
            self._load_padded_f32(b, c, t, dma=dq)
            self.fill_xcols(t, slots=slice(0, 6), engine=nc.scalar)
        PK1 = self.pPK1.tile([P, 6, Wp], dt.uint32, tag="PK1", name="PK1")
        for view, c in zip(self._pk1_views(PK1), PK1_CH):
            stage(c, view)
        self.fill_halos(PK1, engine=nc.gpsimd, dma=nc.gpsimd)
        return {"V0": V0, "V1": V1, "PK1": PK1, "PK2": PK2}

    # ---------- per-iteration ----------

    def phase_A(self, st, thresh_sq):
        nc = self.nc
        vy = _interior(st["V0"])
        vx = _interior(st["V1"])
        E_int = _interior(self.E_view(st["PK2"]))

        # swaps = 8 + (E == 0), derived from E so no swaps halo DMAs are
        # needed. Emitted as interior + xcols + halo-slot pieces: the interior
        # only needs PK2's interior (ready right after the previous gather),
        # so phase A is not blocked on PK2's halo-fill DMAs.
        Ev = self.E_view(st["PK2"])
        swaps = self.pSw.tile([P, 6, Wp], dt.bfloat16, tag="swaps", name="swaps")
        nc.vector.tensor_scalar(out=_interior(swaps), in0=E_int, scalar1=0.0,
                                scalar2=8.0, op0=Alu.is_equal, op1=Alu.add)
        self.fill_xcols(swaps, engine=nc.vector)

        m2 = self.pF.tile([P, S, W], dt.float32, tag="f32t", name="m2")
        _emit_custom(nc, "BV_M2S", m2[:], vy, vx)
        mp = self.pF.tile([P, S, W], dt.float32, tag="f32t", name="mp")
        nc.scalar.activation(mp[:], m2[:], Act.Sqrt)
        nc.vector.scalar_tensor_tensor(out=mp[:], in0=mp[:], scalar=0.002, in1=m2[:],
                                       op0=Alu.mult, op1=Alu.add)      # magp2 - 1e-6

        # swaps halo slots, emitted here so the PK2 halo-fill DMA latency
        # hides behind the m2/magp2 chain (only phase B's first view needs it)
        for sl in (slice(0, 1), slice(5, 6)):
            nc.vector.tensor_scalar(out=swaps[:, sl], in0=Ev[:, sl], scalar1=0.0,
                                    scalar2=8.0, op0=Alu.is_equal, op1=Alu.add)

        def bt(name):
            return self.pT.tile([P, S, W], dt.bfloat16, tag="bft", name=name)

        # en first: it is m2's last consumer, freeing its pF slot before T0/T1
        en = bt("en")
        _emit_custom(nc, "BV_ENW", en[:], m2[:], E_int, s0=float(thresh_sq))
        T0 = self.pF.tile([P, S, W], dt.float32, tag="f32t", name="T0")
        nc.scalar.activation(T0[:], mp[:], Act.Copy, bias=1e-6 * K0SQ, scale=K0SQ)
        T1 = mp  # in place: mp's last reader is T0's activation
        nc.scalar.activation(T1[:], mp[:], Act.Copy, bias=1e-6 * K1SQ, scale=K1SQ)

        s1m = bt("s1m")
        _emit_custom(nc, "BV_S1M", s1m[:], vy, en[:])

        # sector-index field A' instead of 8 one-hot want tiles:
        #   c  = u0+u1+u2+u3  (== |angle| band 0..4, same compare primitives)
        #   A  = (c + s1m*(8-2c)) mod 8        (vy<0 mirrors the sector)
        #   A' = A + 15*(1-en)                 (sentinel >= 15 for gated cells)
        # bf16 arithmetic on small ints is exact, so A' == a replicates the
        # want_a masks bit-for-bit at ~2/3 the DVE cost, and frees the pW pool.
        # band count via the fused CBD custom: c = CBD(T0) + CBD(T1)
        # (bit-identical to the u0+u1+u2+u3 ULE/UGT formulation)
        cb0, cb1 = bt("cb0"), bt("cb1")
        _emit_custom(nc, "BV_CBD", cb0[:], vx, T0[:], s0=2.0)
        _emit_custom(nc, "BV_CBD", cb1[:], vx, T1[:], s0=2.0)
        c = cb0  # in place
        nc.vector.tensor_tensor(out=c[:], in0=cb0[:], in1=cb1[:], op=Alu.add)
        g = cb1  # g = 8 - 2c
        nc.vector.tensor_scalar(out=g[:], in0=c[:], scalar1=-2.0, scalar2=8.0,
                                op0=Alu.mult, op1=Alu.add)
        t4 = bt("t4")  # t4 = s1m * (8-2c)
        nc.vector.tensor_tensor(out=t4[:], in0=s1m[:], in1=g[:], op=Alu.mult)
        A = g  # A = c + t4; the c==0 & vy<0 case gives 8, which must wrap
        # to 0: A <- (A <= 7) * A   (no mod op in the DVE ISA)
        nc.vector.tensor_tensor(out=A[:], in0=c[:], in1=t4[:], op=Alu.add)
        t5 = t4  # t5 = (A <= 7)
        nc.vector.tensor_scalar(out=t5[:], in0=A[:], scalar1=7.0, scalar2=None,
                                op0=Alu.is_le)
        A2 = c  # A2 = A * (A <= 7)
        nc.vector.tensor_tensor(out=A2[:], in0=A[:], in1=t5[:], op=Alu.mult)
        es = s1m  # es = 15*(1-en)
        nc.vector.tensor_scalar(out=es[:], in0=en[:], scalar1=-15.0, scalar2=15.0,
                                op0=Alu.mult, op1=Alu.add)
        nc.vector.tensor_tensor(out=A2[:], in0=A2[:], in1=es[:], op=Alu.add)
        return A2, swaps

    def phase_B(self, Ap, swaps):
        nc = self.nc
        # Per step: M8 = (A' == a) * (view(swaps) == 9) in ONE custom. The
        # mover-side free check (swaps >= 8) is unnecessary because a cell can
        # only mover-match at its own unique sector step; cells claimed as
        # TARGETS get A' invalidated (set to 15) with the same view-mask CP
        # that writes swaps, so they can never mover-match later.
        for a in range(8):
            dy, dx = _DY[a], _DX[a]
            a4 = (a + 4) % 8
            dy4, dx4 = -dy, -dx
            M8 = self.pM8.tile([P, 6, Wp], dt.uint16, tag="M8", name="M8")
            _emit_custom(nc, "BV_EQE", _interior(M8), Ap[:], _view(swaps, dy, dx),
                         s0=float(a), s1=9.0)
            # halos of M8 on the (dy4, dx4) view side, bulk split sync/scalar
            self.fill_xcol_side(M8, dx4, engine=nc.vector)
            if dy4 > 0:
                self.fill_yhalo(M8, hi=True, dma=nc.sync, wrap_dma=nc.gpsimd,
                                dma2=nc.scalar)
            elif dy4 < 0:
                self.fill_yhalo(M8, hi=False, dma=nc.sync, wrap_dma=nc.gpsimd,
                                dma2=nc.scalar)
            nc.vector.copy_predicated(out=_interior(swaps), mask=_interior(M8),
                                      data=self.cval(a))
            nc.vector.copy_predicated(out=_interior(swaps), mask=_view(M8, dy4, dx4),
                                      data=self.cval(a4))
            if a < 7:
                nc.vector.copy_predicated(out=Ap[:], mask=_view(M8, dy4, dx4),
                                          data=self.cval(15))
            # refresh swaps halos needed by the next step's view
            if a < 7:
                dyn, dxn = _DY[a + 1], _DX[a + 1]
                self.fill_xcol_side(swaps, dxn, engine=nc.vector)
                if dyn > 0:
                    self.fill_yhalo(swaps, hi=True, dma=nc.sync,
                                    wrap_dma=nc.gpsimd, dma2=nc.scalar)
                elif dyn < 0:
                    self.fill_yhalo(swaps, hi=False, dma=nc.sync,
                                    wrap_dma=nc.gpsimd, dma2=nc.scalar)
        return swaps

    def phase_C_prep(self, st):
        """Allocate + init the next-state tiles. Emitted BEFORE phase B so the
        Pool copies run during B (in-order queues would otherwise pin them
        behind B's per-step xcol ops)."""
        nc = self.nc
        V0, V1, PK1, PK2 = st["V0"], st["V1"], st["PK1"], st["PK2"]
        nV0 = self.pV.tile([P, 6, Wp], dt.float32, tag="Vp", name="nV0")
        nV1 = self.pV.tile([P, 6, Wp], dt.float32, tag="Vp", name="nV1")
        nPK1 = self.pPK1.tile([P, 6, Wp], dt.uint32, tag="PK1", name="nPK1")
        nPK2 = self.pPK2.tile([P, 6, Wp], dt.uint16, tag="PK2", name="nPK2")
        # nPK1 MUST be bit-exact: u32-packed fp8 bytes through the ACT f32
        # datapath lose low mantissa bits (ch1/ch2 corruption) - DMA it.
        # u16 (< 2^24) and f32 round-trip exactly through ACT.
        nc.sync.dma_start(out=_interior(nPK1), in_=_interior(PK1))
        nc.scalar.copy(out=_interior(nPK2), in_=_interior(PK2))
        nc.scalar.copy(out=_interior(nV0), in_=_interior(V0))
        nc.scalar.copy(out=_interior(nV1), in_=_interior(V1))
        return nV0, nV1, nPK1, nPK2

    def phase_C(self, st, swaps, news, last):
        nc = self.nc
        V0, V1, PK1, PK2 = st["V0"], st["V1"], st["PK1"], st["PK2"]
        nV0, nV1, nPK1, nPK2 = news
        for a in range(8):
            dy, dx = _DY[a], _DX[a]
            equ = self.pB.tile([P, S, W], dt.uint16, tag="bstep", name="equ")
            nc.vector.tensor_scalar(out=equ[:], in0=_interior(swaps), scalar1=float(a),
                                    scalar2=None, op0=Alu.is_equal)
            nc.vector.copy_predicated(out=_interior(nPK2), mask=equ[:],
                                      data=_view(PK2, dy, dx))
            nc.vector.copy_predicated(out=_interior(nPK1), mask=equ[:],
                                      data=_view(PK1, dy, dx))
            nc.vector.copy_predicated(out=_interior(nV0), mask=equ[:],
                                      data=_view(V0, dy, dx))
            nc.vector.copy_predicated(out=_interior(nV1), mask=equ[:],
                                      data=_view(V1, dy, dx))
        if not last:
            # PK halos issued BEFORE the blend: their DMAs fly while the DVE
            # does the blend, so the next iteration's swaps halo-slot init
            # (which reads PK2's halo) doesn't stall. PK2 gathers/fills come
            # first - they are the most urgent.
            self.fill_halos_yfirst(nPK2)
            self.fill_halos_yfirst(nPK1)
        # vel blend: nV = 0.5*(nV + V) on Vector (Pool/ACT variants measured
        # slower end-to-end: the blend sits on the iteration's critical tail).
        # On the last iteration the final vel *= 0.95 decay is folded in.
        bs = 0.475 if last else 0.5
        for nV, V in ((nV0, V0), (nV1, V1)):
            nc.vector.tensor_tensor(out=_interior(nV), in0=_interior(nV),
                                    in1=_interior(V), op=Alu.add)
            nc.vector.tensor_scalar(out=_interior(nV), in0=_interior(nV),
                                    scalar1=bs, scalar2=None, op0=Alu.mult)
        st["V0"], st["V1"], st["PK1"], st["PK2"] = nV0, nV1, nPK1, nPK2
        if not last:
            # V halos aren't read until the next phase C - fill last
            self.fill_halos(nV0, engine=nc.scalar, dma=nc.sync, wrap_dma=nc.gpsimd,
                            dma2=nc.scalar)
            self.fill_halos(nV1, engine=nc.scalar, dma=nc.sync, wrap_dma=nc.gpsimd,
                            dma2=nc.scalar)

    def image_iter(self, st, n):
        thresh_sq = 1.0 if n == 0 else 4.0
        Ap, swaps = self.phase_A(st, thresh_sq)
        news = self.phase_C_prep(st)
        swaps = self.phase_B(Ap, swaps)
        self.phase_C(st, swaps, news, last=(n == 1))

    # ---------- final conv + stores ----------

    def image_final(self, b, st):
        nc = self.nc
        PK1, PK2 = st["PK1"], st["PK2"]
        nk = self.nk
        uniform = bool(np.allclose(nk, nk[0, 0]))
        assert uniform, "non-uniform neighbor_kernel not supported in this build"
        scale = float(nk[0, 0])

        # payload stores FIRST: PK planes are final right after phase C, and
        # storing early releases their pool buffers for the next image's prep
        all_views = list(zip(self._pk1_views(PK1), PK1_CH)) + \
                    list(zip(self._pk2_views(PK2), PK2_CH))
        for i, (view, c) in enumerate(all_views):
            stg = self.pV.tile([P, S, W], dt.float32, tag="Vp", name="ostg")
            if i % 2 == 0:
                nc.scalar.copy(out=stg[:], in_=view[:, 1:1 + S, 1:1 + W])
            else:
                nc.vector.tensor_copy(out=stg[:], in_=view[:, 1:1 + S, 1:1 + W])
            dq = (nc.gpsimd, nc.scalar)[i % 2]
            dq.dma_start(out=self.wout[b, c].rearrange("(p k) x -> p k x", k=S),
                         in_=stg[:])

        for c, key in ((3, "V0"), (4, "V1")):
            V = st[key]
            # (vel *= 0.95 is folded into the last blend as 0.475)
            nc.vector.memset(V[:, 1:5, 0:1], 0)
            nc.vector.memset(V[:, 1:5, Wp - 1:Wp], 0)
            self.fill_yhalo(V, hi=True, zero_edge=True, dma=nc.sync,
                            dma2=nc.scalar)
            self.fill_yhalo(V, hi=False, zero_edge=True, dma=nc.sync,
                            dma2=nc.scalar)
            # row sums over x into a padded tmp; zero y-edges; column sums
            tp = self.pV.tile([P, 6, Wp], dt.float32, tag="Vp", name="convtp")
            # row sums computed over ALL 6 slots (halo rows included): V's
            # halo slots hold valid (zero-edged) neighbor rows, so tp's halo
            # rows come out bit-identical to a DMA'd copy of the neighbor's
            # tp - and two DMA chains leave the final's critical tail.
            nc.gpsimd.tensor_tensor(out=tp[:, 0:6, 1:1 + W],
                                    in0=V[:, 0:6, 0:W],
                                    in1=V[:, 0:6, 1:1 + W], op=Alu.add)
            nc.vector.tensor_tensor(out=tp[:, 0:6, 1:1 + W],
                                    in0=tp[:, 0:6, 1:1 + W],
                                    in1=V[:, 0:6, 2:2 + W], op=Alu.add)
            acc = self.pF.tile([P, S, W], dt.float32, tag="f32t", name="acc")
            nc.gpsimd.tensor_tensor(out=acc[:], in0=_view(tp, -1, 0),
                                    in1=_view(tp, 0, 0), op=Alu.add)
            nc.vector.tensor_tensor(out=acc[:], in0=acc[:], in1=_view(tp, 1, 0), op=Alu.add)
            half = self.pF.tile([P, S, W], dt.float32, tag="f32t", name="half")
            nc.scalar.mul(half[:], _interior(V), 0.5)
            nc.vector.scalar_tensor_tensor(out=acc[:], in0=acc[:], scalar=scale,
                                           in1=half[:], op0=Alu.mult, op1=Alu.add)
            nc.scalar.dma_start(out=self.wout[b, c].rearrange("(p k) x -> p k x", k=S),
                                in_=acc[:])


def _build(nk):
    return _Emit(nk).build()


def kernel(world, rand_movement=None, rand_interact=None, rand_element=None,
           neighbor_kernel=None, **_kw):
    world = np.ascontiguousarray(np.asarray(world, dtype=np.float32))
    nk = np.asarray(neighbor_kernel, dtype=np.float32).reshape(3, 3) / 18.0
    key = nk.tobytes()
    nc = _cache.get(key)
    if nc is None:
        nc = _cache[key] = _build(nk)
    in_maps = [{"w": world[NB * i:NB * (i + 1)]} for i in range(NCORES)]
    res = run_bass_kernel_spmd(nc, in_maps, list(range(NCORES))).results
    return np.concatenate([r["o"] for r in res], axis=0)

